# revision 9
# baseline (speedup 1.0000x reference)
"""DeepseekOCR text MoE layer on 8 Trainium2 NeuronCores.

Expert-parallel: 4 routed experts per core (bucketed by token count so
every core's slot j has a similar load); shared expert tensor-sharded
over its intermediate dim (352 columns per core). Router + token
gather/scatter run on host (full-I/O contract).

Precision split (validated on host: rel err ~1.0e-2 vs fp32 reference):
  * routed experts run in fp8e4 with DoubleRow matmuls (2 contraction
    rows per PE cell -> ~1.7x PE throughput). Scales: x unscaled,
    wg/wu x8 (PSUM = 8*g, silu descales by 1/8, so hT = 8*h lands in
    fp8 range directly), wd x128; host divides the combine weights by
    8*128.
  * shared expert stays bf16 (it carries ~98% of the output norm, so
    fp8 there would blow the error budget).

Device program per core:
  phase A (per expert slot):  hT[h,c] = silu(wg.T @ xgT) * (wu.T @ xgT)
  phase B (per expert slot):  yT[d,c] = wd.T-tiles @ hT   (tokens on the
                              moving free dim -> no 128-token rounding)
  shared (per 512-token quarter): same silu-mlp with sharded weights.
Host: out = scatter_add(yT * combine_w) + sum_cores(ys).
"""

import numpy as np
import ml_dtypes

import concourse.bacc as bacc
import concourse.mybir as mybir
import concourse.tile as tile
from concourse.bass_utils import run_bass_kernel_spmd

B, S, D = 2, 1024, 2048
E, H, K = 32, 1408, 6
H_SHARED = 2816
ROUTED_SCALE = 1.0
T = B * S                      # 2048 tokens
N_CORES = 8
E_LOC = E // N_CORES           # 4 experts per core
HS_LOC = H_SHARED // N_CORES   # 352 shared-intermediate cols per core
HS_PAD = 384                   # padded to 3 k-tiles of 128
NH = H // 128                  # 11 h-tiles per routed expert
ND = D // 512                  # 4 d-groups (512 cols each)
NKD = D // 128                 # 16 contraction k-tiles over D
NKP = NKD // 2                 # 8 DoubleRow k-pairs over D
NHP = NH // 2                  # 5 DoubleRow h-pairs (plus 1 odd tile)
NSH = HS_PAD // 128            # 3 h-tiles for shared
TQ = 512                       # shared-expert token chunk
NTQ = T // TQ                  # 4 chunks
NXG = 2                        # xg split into k-chunks for early start
SWA = 8.0                      # gate/up weight scale (fp8)
SWD = 128.0                    # down weight scale (fp8)
YDIV = SWA * SWD               # host-side descale of routed outputs

BF16 = ml_dtypes.bfloat16
F8 = ml_dtypes.float8_e4m3
f32 = mybir.dt.float32
bf16 = mybir.dt.bfloat16
fp8 = mybir.dt.float8e4
DR = mybir.MatmulPerfMode.DoubleRow

LAST_RESULTS = None            # BassKernelResults of the latest run (for test harness)


def _route(x, gate_w):
    """Greedy top-k softmax router, fp32 numpy (matches jax.lax.top_k order)."""
    logits = x @ gate_w.T                              # [T, E]
    m = logits.max(-1, keepdims=True)
    ex = np.exp(logits - m)
    scores = ex / ex.sum(-1, keepdims=True)
    topk_i = np.argsort(-scores, axis=-1, kind="stable")[:, :K]
    topk_w = np.take_along_axis(scores, topk_i, -1) * ROUTED_SCALE
    return topk_i, topk_w.astype(np.float32)


def _expert_mlp(nc, pools, slabs, C, hT_tag, nh):
    """Emit phase A (gate/up + silu*mul -> hT, fp8 DoubleRow) for one expert."""
    psA, tmp_p, ht_p = pools
    xg_chunks, w_slabs = slabs                  # w_slabs[h] = (gate_slab, up_slab)
    NCC = -(-C // 512)
    hT = ht_p.tile([128, nh, C], fp8, tag=hT_tag)
    KC = NKD // NXG
    KPC = KC // 2                               # DR k-pairs per xg chunk
    for h in range(nh):
        wg_s, wu_s = w_slabs[h]
        for cc in range(NCC):
            w = min(512, C - cc * 512)
            cs = slice(cc * 512, cc * 512 + w)
            pg = psA.tile([128, w], f32, tag="psA")
            for kk in range(NKP):
                mv = xg_chunks[kk // KPC][:, 2 * (kk % KPC):2 * (kk % KPC) + 2, cs]
                nc.tensor.matmul(pg[:], wg_s[:, 2 * kk:2 * kk + 2, :], mv,
                                 start=(kk == 0), stop=(kk == NKP - 1),
                                 perf_mode=DR)
            pu = psA.tile([128, w], f32, tag="psA")
            for kk in range(NKP):
                mv = xg_chunks[kk // KPC][:, 2 * (kk % KPC):2 * (kk % KPC) + 2, cs]
                nc.tensor.matmul(pu[:], wu_s[:, 2 * kk:2 * kk + 2, :], mv,
                                 start=(kk == 0), stop=(kk == NKP - 1),
                                 perf_mode=DR)
            tmp = tmp_p.tile([128, 512], bf16, tag="tmp")
            nc.scalar.activation(tmp[:, :w], pg[:],
                                 mybir.ActivationFunctionType.Silu,
                                 scale=1.0 / SWA)
            nc.vector.tensor_mul(hT[:, h, cs], tmp[:, :w], pu[:])
    return hT


def _build_bass(Cs):
    """Per-core Tile program; Cs[j] = routed token capacity of expert slot j."""
    Cmax = max(Cs)
    nc = bacc.Bacc(None, target_bir_lowering=False)

    xgt_js = [nc.dram_tensor(f"xgt{j}", [128, NKD, Cs[j]], fp8, kind="ExternalInput")
              for j in range(E_LOC)]
    wgu = nc.dram_tensor("wgu", [E_LOC, NH, 128, 2, NKD, 128], fp8, kind="ExternalInput")
    wdd = nc.dram_tensor("wdd", [E_LOC, 2 * ND, 128, NH, 2, 128], fp8, kind="ExternalInput")
    xtq = nc.dram_tensor("xtq", [NTQ, 128, NKD, TQ], bf16, kind="ExternalInput")
    swgu = nc.dram_tensor("swgu", [128, 2, NSH, NKD, 128], bf16, kind="ExternalInput")
    swdd = nc.dram_tensor("swdd", [128, ND, NSH, 512], bf16, kind="ExternalInput")
    y_outs = [nc.dram_tensor(f"y_out{j}", [ND * 4, 128, Cs[j]], bf16,
                             kind="ExternalOutput") for j in range(E_LOC)]
    ys_out = nc.dram_tensor("ys_out", [T // 128, ND, 128, 512], bf16, kind="ExternalOutput")

    with tile.TileContext(nc) as tc:
        with (
            tc.tile_pool(name="wgu_p", bufs=12) as wgu_p,
            tc.tile_pool(name="wd_p", bufs=4) as wd_p,
            tc.tile_pool(name="swgu_p", bufs=1) as swgu_p,
            tc.tile_pool(name="swd_p", bufs=1) as swd_p,
            tc.tile_pool(name="xg_p", bufs=2 * NXG) as xg_p,
            tc.tile_pool(name="xt_p", bufs=3) as xt_p,
            tc.tile_pool(name="ht_p", bufs=2) as ht_p,
            tc.tile_pool(name="hst_p", bufs=2) as hst_p,
            tc.tile_pool(name="tmp_p", bufs=2) as tmp_p,
            tc.tile_pool(name="y_p", bufs=4) as y_p,
            tc.tile_pool(name="psA", bufs=4, space="PSUM") as psA,
            tc.tile_pool(name="psB", bufs=4, space="PSUM") as psB,
        ):
            KC = NKD // NXG
            sg_slabs, sd_slabs, xq_tiles = [], [], [None] * NTQ

            # PE warm-up on zeros while the first loads land (HAM un-throttle)
            warm = tmp_p.tile([128, 512], bf16, tag="tmp")
            nc.vector.memset(warm[:], 0.0)
            pwarm = psA.tile([128, 512], f32, tag="psA")
            for _ in range(14):
                nc.tensor.matmul(pwarm[:], warm[:, :128], warm[:], start=True, stop=True)

            def load_shared():
                # ordered by first use: h0 gate/up + xq0 first, then the
                # rest of swgu, then swdd (needed ~15us into quarter 0)
                s = swgu_p.tile([128, 2, NSH, NKD, 128], bf16, tag="swgu")
                nc.sync.dma_start(s[:, 0, 0], swgu[:, 0, 0])
                nc.sync.dma_start(s[:, 1, 0], swgu[:, 1, 0])
                sg_slabs.append(s)
                xq_tiles[0] = xt_p.tile([128, NKD, TQ], bf16, tag="xt", name="xq0")
                nc.sync.dma_start(xq_tiles[0][:], xtq[0])
                for h in range(1, NSH):
                    nc.sync.dma_start(s[:, 0, h], swgu[:, 0, h])
                    nc.sync.dma_start(s[:, 1, h], swgu[:, 1, h])
                s2 = swd_p.tile([128, ND, NSH, 512], bf16, tag="swd")
                nc.sync.dma_start(s2[:], swdd[:])
                sd_slabs.append(s2)

            def shared_quarter(q):
                """One 512-token slice of the shared expert (bf16)."""
                st_eng = nc.sync if q == NTQ - 1 else nc.scalar
                xq = xq_tiles[q]
                if q + 1 < NTQ and xq_tiles[q + 1] is None:
                    xq_tiles[q + 1] = xt_p.tile([128, NKD, TQ], bf16, tag="xt", name=f"xq{q+1}")
                    nc.sync.dma_start(xq_tiles[q + 1][:], xtq[q + 1])
                hsT = hst_p.tile([128, NSH, TQ], bf16, tag="hst")
                sgu = sg_slabs[0]
                for h in range(NSH):
                    pg = psA.tile([128, TQ], f32, tag="psA")
                    for k in range(NKD):
                        nc.tensor.matmul(pg[:], sgu[:, 0, h, k], xq[:, k],
                                         start=(k == 0), stop=(k == NKD - 1))
                    pu = psA.tile([128, TQ], f32, tag="psA")
                    for k in range(NKD):
                        nc.tensor.matmul(pu[:], sgu[:, 1, h, k], xq[:, k],
                                         start=(k == 0), stop=(k == NKD - 1))
                    tmp = tmp_p.tile([128, 512], bf16, tag="tmp")
                    nc.scalar.activation(tmp[:, :TQ], pg[:],
                                         mybir.ActivationFunctionType.Silu)
                    nc.vector.tensor_mul(hsT[:, h, :], tmp[:, :TQ], pu[:])
                for d in range(ND):
                    for ci in range(TQ // 128):
                        py = psB.tile([128, 512], f32, tag="psB")
                        for h in range(NSH):
                            nc.tensor.matmul(py[:], hsT[:, h, ci * 128:(ci + 1) * 128],
                                             sd_slabs[0][:, d, h],
                                             start=(h == 0), stop=(h == NSH - 1))
                        yst = y_p.tile([128, 512], bf16, tag="y")
                        nc.vector.tensor_copy(yst[:], py[:])
                        st_eng.dma_start(ys_out[q * 4 + ci, d], yst[:])

            # ---- routed experts, shared quarters interleaved as DMA slack ----
            def load_xg(j):
                chunks = []
                for g in range(NXG):
                    xc = xg_p.tile([128, KC, Cs[j]], fp8, tag="xg", name=f"xg{j}_{g}")
                    nc.sync.dma_start(xc[:], xgt_js[j][:, g * KC:(g + 1) * KC, :])
                    chunks.append(xc)
                return chunks

            def load_wgu_h0(j):
                gu = wgu_p.tile([128, 2, NKD, 128], fp8, tag="wgu", name=f"wgu{j}_h0")
                nc.sync.dma_start(gu[:, 0], wgu[j, 0, :, 0])
                nc.sync.dma_start(gu[:, 1], wgu[j, 0, :, 1])
                return gu

            def load_wgu_rest(j, gu0):
                slabs = [(gu0[:, 0], gu0[:, 1])]
                for h in range(1, NH):
                    gu = wgu_p.tile([128, 2, NKD, 128], fp8, tag="wgu")
                    nc.sync.dma_start(gu[:], wgu[j, h])
                    slabs.append((gu[:, 0], gu[:, 1]))
                return slabs

            xg_next = None
            slabs_next = None
            for j in range(E_LOC):
                C = Cs[j]
                if j == 0:
                    gu0 = load_wgu_h0(0)
                    xg_next = load_xg(0)
                    slabs_next = load_wgu_rest(0, gu0)
                w_slabs = slabs_next
                xg_chunks = xg_next
                hT = _expert_mlp(nc, (psA, tmp_p, ht_p),
                                 (xg_chunks, w_slabs), C, "ht", NH)
                # phase B': stationary = wd d-tiles (fp8 DoubleRow over h-pairs),
                # moving = hT tokens
                NCC = -(-C // 512)
                st_eng = nc.sync if j == E_LOC - 1 else nc.scalar
                for dh in range(2 * ND):
                    wd_s = wd_p.tile([128, NH, 2, 128], fp8, tag="wd")
                    nc.sync.dma_start(wd_s[:], wdd[j, dh])
                    for dt in range(2):
                        for cc in range(NCC):
                            w = min(512, C - cc * 512)
                            cs = slice(cc * 512, cc * 512 + w)
                            py = psB.tile([128, w], f32, tag="psB")
                            for hh in range(NHP):
                                nc.tensor.matmul(py[:],
                                                 wd_s[:, 2 * hh:2 * hh + 2, dt, :],
                                                 hT[:, 2 * hh:2 * hh + 2, cs],
                                                 start=(hh == 0), stop=False,
                                                 perf_mode=DR)
                            nc.tensor.matmul(py[:], wd_s[:, NH - 1, dt, :],
                                             hT[:, NH - 1, cs],
                                             start=False, stop=True)
                            yst = y_p.tile([128, 512], bf16, tag="y")
                            nc.vector.tensor_copy(yst[:, :w], py[:])
                            st_eng.dma_start(
                                y_outs[j][dh * 2 + dt, :, cs], yst[:, :w])
                if j == 0:
                    load_shared()
                if j + 1 < E_LOC:
                    xg_next = load_xg(j + 1)
                    gu0 = load_wgu_h0(j + 1)
                    slabs_next = load_wgu_rest(j + 1, gu0)
                shared_quarter(j)
    nc.compile()
    return nc


def kernel(hidden_states, gate_w, wg, wu, wd, swg, swu, swd):
    global LAST_RESULTS
    x = np.ascontiguousarray(np.asarray(hidden_states, np.float32).reshape(T, D))
    gate_w = np.asarray(gate_w, np.float32)
    wg = np.asarray(wg, np.float32)
    wu = np.asarray(wu, np.float32)
    wd = np.asarray(wd, np.float32)
    swg = np.asarray(swg, np.float32)
    swu = np.asarray(swu, np.float32)
    swd = np.asarray(swd, np.float32)

    # ---- host router ----
    topk_i, topk_w = _route(x, gate_w)
    idx = [np.where((topk_i == e).any(-1))[0] for e in range(E)]
    wts = [(topk_w * (topk_i == e))[idx[e]].sum(-1).astype(np.float32) / YDIV
           for e in range(E)]
    cnts = np.array([len(i) for i in idx])
    # bucket experts: slot j on every core serves similarly-loaded experts
    ranked = np.argsort(-cnts, kind="stable")            # expert ids, busiest first
    emap = ranked.reshape(E_LOC, N_CORES)                # emap[j, c] -> expert id
    Cs = [max(16, -(-int(cnts[emap[j]].max()) // 16) * 16) for j in range(E_LOC)]
    Cmax = max(Cs)

    nc = _build_bass(Cs)

    # ---- host shard + layout prep (all DMA sources partition-major) ----
    xT = np.ascontiguousarray(x.T)                      # [D, T] fp32
    xtq_np = np.ascontiguousarray(
        xT.reshape(NKD, 128, NTQ, TQ).transpose(2, 1, 0, 3).astype(BF16))

    in_maps = []
    for c in range(N_CORES):
        wgu_np = np.empty((E_LOC, NH, 128, 2, NKD, 128), F8)
        wdd_np = np.empty((E_LOC, 2 * ND, 128, NH, 2, 128), F8)
        xgt_nps = [np.zeros((128, NKD, Cs[j]), F8) for j in range(E_LOC)]
        for j in range(E_LOC):
            e = int(emap[j, c])
            wgu_np[j] = ((np.stack([wg[e], wu[e]]) * SWA)
                         .reshape(2, NKD, 128, NH, 128)
                         .transpose(3, 2, 0, 1, 4).astype(F8))
            wdd_np[j] = ((wd[e] * SWD).reshape(NH, 128, 2 * ND, 2, 128)
                         .transpose(2, 1, 0, 3, 4).astype(F8))
            cnt = int(cnts[e])
            xg = xT[:, idx[e]]                          # [D, cnt] fp32
            xgt_nps[j][:, :, :cnt] = (xg.reshape(NKD, 128, cnt)
                                      .transpose(1, 0, 2).astype(F8))
        sl = slice(c * HS_LOC, (c + 1) * HS_LOC)
        swg_c = np.zeros((D, HS_PAD), np.float32); swg_c[:, :HS_LOC] = swg[:, sl]
        swu_c = np.zeros((D, HS_PAD), np.float32); swu_c[:, :HS_LOC] = swu[:, sl]
        swd_c = np.zeros((HS_PAD, D), np.float32); swd_c[:HS_LOC] = swd[sl, :]
        swgu_np = (np.stack([swg_c, swu_c])
                   .reshape(2, NKD, 128, NSH, 128)
                   .transpose(2, 0, 3, 1, 4).astype(BF16))
        swdd_np = (swd_c.reshape(NSH, 128, ND, 512)
                   .transpose(1, 2, 0, 3).astype(BF16))
        im = {f"xgt{j}": xgt_nps[j] for j in range(E_LOC)}
        in_maps.append(im)
        im.update({
            "wgu": np.ascontiguousarray(wgu_np),
            "wdd": np.ascontiguousarray(wdd_np),
            "xtq": xtq_np,
            "swgu": np.ascontiguousarray(swgu_np),
            "swdd": np.ascontiguousarray(swdd_np),
        })

    res = run_bass_kernel_spmd(nc, in_maps, core_ids=list(range(N_CORES)))
    LAST_RESULTS = res

    # ---- host unshard: scatter-add routed outputs, sum shared partials ----
    out = np.zeros((T, D), np.float32)
    for c in range(N_CORES):
        ys = res.results[c]["ys_out"].astype(np.float32)  # [16, ND, 128, 512]
        out += ys.transpose(0, 2, 1, 3).reshape(T, D)
        for j in range(E_LOC):
            e = int(emap[j, c])
            cnt = int(cnts[e])
            y = res.results[c][f"y_out{j}"].reshape(D, Cs[j])[:, :cnt].astype(np.float32)
            out[idx[e]] += (y * wts[e][None, :]).T
    return out.reshape(B, S, D)


# revision 10
# speedup vs baseline: 1.0450x; 1.0450x over previous
"""DeepseekOCR text MoE layer on 8 Trainium2 NeuronCores.

Expert-parallel: 4 routed experts per core (bucketed by token count so
every core's slot j has a similar load); shared expert tensor-sharded
over its intermediate dim (352 columns per core). Router + token
gather/scatter run on host (full-I/O contract).

Precision split (validated on host: rel err ~1.0e-2 vs fp32 reference):
  * routed experts run in fp8e4 with DoubleRow matmuls (2 contraction
    rows per PE cell -> ~1.7x PE throughput). Scales: x unscaled,
    wg/wu x8 (PSUM = 8*g, silu descales by 1/8, so hT = 8*h lands in
    fp8 range directly), wd x128; host divides the combine weights by
    8*128.
  * shared expert stays bf16 (it carries ~98% of the output norm, so
    fp8 there would blow the error budget).

Device program per core:
  phase A (per expert slot):  hT[h,c] = silu(wg.T @ xgT) * (wu.T @ xgT)
  phase B (per expert slot):  yT[d,c] = wd.T-tiles @ hT   (tokens on the
                              moving free dim -> no 128-token rounding)
  shared (per 512-token quarter): same silu-mlp with sharded weights.
Host: out = scatter_add(yT * combine_w) + sum_cores(ys).
"""

import numpy as np
import ml_dtypes

import concourse.bacc as bacc
import concourse.mybir as mybir
import concourse.tile as tile
from concourse.bass_utils import run_bass_kernel_spmd

B, S, D = 2, 1024, 2048
E, H, K = 32, 1408, 6
H_SHARED = 2816
ROUTED_SCALE = 1.0
T = B * S                      # 2048 tokens
N_CORES = 8
E_LOC = E // N_CORES           # 4 experts per core
HS_LOC = H_SHARED // N_CORES   # 352 shared-intermediate cols per core
HS_PAD = 384                   # padded to 3 k-tiles of 128
NH = H // 128                  # 11 h-tiles per routed expert
ND = D // 512                  # 4 d-groups (512 cols each)
NKD = D // 128                 # 16 contraction k-tiles over D
NKP = NKD // 2                 # 8 DoubleRow k-pairs over D
NHP = NH // 2                  # 5 DoubleRow h-pairs (plus 1 odd tile)
NSH = HS_PAD // 128            # 3 h-tiles for shared
TQ = 512                       # shared-expert token chunk
NTQ = T // TQ                  # 4 chunks
NXG = 2                        # xg split into k-chunks for early start
SWA = 8.0                      # gate/up weight scale (fp8)
SWD = 128.0                    # down weight scale (fp8)
YDIV = SWA * SWD               # host-side descale of routed outputs

BF16 = ml_dtypes.bfloat16
F8 = ml_dtypes.float8_e4m3
f32 = mybir.dt.float32
bf16 = mybir.dt.bfloat16
fp8 = mybir.dt.float8e4
DR = mybir.MatmulPerfMode.DoubleRow

LAST_RESULTS = None            # BassKernelResults of the latest run (for test harness)


def _route(x, gate_w):
    """Greedy top-k softmax router, fp32 numpy (matches jax.lax.top_k order)."""
    logits = x @ gate_w.T                              # [T, E]
    m = logits.max(-1, keepdims=True)
    ex = np.exp(logits - m)
    scores = ex / ex.sum(-1, keepdims=True)
    topk_i = np.argsort(-scores, axis=-1, kind="stable")[:, :K]
    topk_w = np.take_along_axis(scores, topk_i, -1) * ROUTED_SCALE
    return topk_i, topk_w.astype(np.float32)


def _expert_mlp(nc, pools, slabs, C, hT_tag, nh):
    """Emit phase A (gate/up + silu*mul -> hT, fp8 DoubleRow) for one expert."""
    psA, tmp_p, ht_p = pools
    xg_chunks, w_slabs = slabs                  # w_slabs[h] = (gate_slab, up_slab)
    NCC = -(-C // 512)
    hT = ht_p.tile([128, nh, C], fp8, tag=hT_tag)
    KC = NKD // NXG
    KPC = KC // 2                               # DR k-pairs per xg chunk
    for h in range(nh):
        wg_s, wu_s = w_slabs[h]
        for cc in range(NCC):
            w = min(512, C - cc * 512)
            cs = slice(cc * 512, cc * 512 + w)
            pg = psA.tile([128, w], f32, tag="psA")
            for kk in range(NKP):
                mv = xg_chunks[kk // KPC][:, 2 * (kk % KPC):2 * (kk % KPC) + 2, cs]
                nc.tensor.matmul(pg[:], wg_s[:, 2 * kk:2 * kk + 2, :], mv,
                                 start=(kk == 0), stop=(kk == NKP - 1),
                                 perf_mode=DR)
            pu = psA.tile([128, w], f32, tag="psA")
            for kk in range(NKP):
                mv = xg_chunks[kk // KPC][:, 2 * (kk % KPC):2 * (kk % KPC) + 2, cs]
                nc.tensor.matmul(pu[:], wu_s[:, 2 * kk:2 * kk + 2, :], mv,
                                 start=(kk == 0), stop=(kk == NKP - 1),
                                 perf_mode=DR)
            tmp = tmp_p.tile([128, 512], bf16, tag="tmp")
            nc.scalar.activation(tmp[:, :w], pg[:],
                                 mybir.ActivationFunctionType.Silu,
                                 scale=1.0 / SWA)
            nc.vector.tensor_mul(hT[:, h, cs], tmp[:, :w], pu[:])
    return hT


def _build_bass(Cs):
    """Per-core Tile program; Cs[j] = routed token capacity of expert slot j."""
    Cmax = max(Cs)
    nc = bacc.Bacc(None, target_bir_lowering=False)

    xgt_js = [nc.dram_tensor(f"xgt{j}", [128, NKD, Cs[j]], fp8, kind="ExternalInput")
              for j in range(E_LOC)]
    wgu = nc.dram_tensor("wgu", [E_LOC, NH, 128, 2, NKD, 128], fp8, kind="ExternalInput")
    wdd = nc.dram_tensor("wdd", [E_LOC, 2 * ND, 128, NH, 2, 128], fp8, kind="ExternalInput")
    xtq = nc.dram_tensor("xtq", [NTQ, 128, NKD, TQ], bf16, kind="ExternalInput")
    swgu = nc.dram_tensor("swgu", [128, 2, NSH, NKD, 128], bf16, kind="ExternalInput")
    swdd = nc.dram_tensor("swdd", [128, ND, NSH, 512], bf16, kind="ExternalInput")
    y_outs = [nc.dram_tensor(f"y_out{j}", [ND * 4, 128, Cs[j]], bf16,
                             kind="ExternalOutput") for j in range(E_LOC)]
    ys_out = nc.dram_tensor("ys_out", [T // 128, ND, 128, 512], bf16, kind="ExternalOutput")

    with tile.TileContext(nc) as tc:
        with (
            tc.tile_pool(name="wgu_p", bufs=12) as wgu_p,
            tc.tile_pool(name="wd_p", bufs=4) as wd_p,
            tc.tile_pool(name="swgu_p", bufs=1) as swgu_p,
            tc.tile_pool(name="swd_p", bufs=1) as swd_p,
            tc.tile_pool(name="xg_p", bufs=2 * NXG) as xg_p,
            tc.tile_pool(name="xt_p", bufs=3) as xt_p,
            tc.tile_pool(name="ht_p", bufs=2) as ht_p,
            tc.tile_pool(name="hst_p", bufs=2) as hst_p,
            tc.tile_pool(name="tmp_p", bufs=2) as tmp_p,
            tc.tile_pool(name="y_p", bufs=8) as y_p,
            tc.tile_pool(name="psA", bufs=4, space="PSUM") as psA,
            tc.tile_pool(name="psB", bufs=4, space="PSUM") as psB,
        ):
            KC = NKD // NXG
            sg_slabs, sd_slabs, xq_tiles = [], [], [None] * NTQ

            # PE warm-up on zeros while the first loads land (HAM un-throttle)
            warm = tmp_p.tile([128, 512], bf16, tag="tmp")
            nc.vector.memset(warm[:], 0.0)
            pwarm = psA.tile([128, 512], f32, tag="psA")
            for _ in range(14):
                nc.tensor.matmul(pwarm[:], warm[:, :128], warm[:], start=True, stop=True)

            def load_shared():
                # head-critical pieces ride the scalar ring (idle until the
                # first y stores); the rest stream on sync behind wd slabs
                s = swgu_p.tile([128, 2, NSH, NKD, 128], bf16, tag="swgu")
                nc.scalar.dma_start(s[:, 0, 0], swgu[:, 0, 0])
                nc.scalar.dma_start(s[:, 1, 0], swgu[:, 1, 0])
                sg_slabs.append(s)
                xq_tiles[0] = xt_p.tile([128, NKD, TQ], bf16, tag="xt", name="xq0")
                nc.scalar.dma_start(xq_tiles[0][:], xtq[0])
                for h in range(1, NSH):
                    nc.scalar.dma_start(s[:, 0, h], swgu[:, 0, h])
                    nc.scalar.dma_start(s[:, 1, h], swgu[:, 1, h])
                s2 = swd_p.tile([128, ND, NSH, 512], bf16, tag="swd")
                nc.sync.dma_start(s2[:], swdd[:])
                sd_slabs.append(s2)

            def shared_quarter(q):
                """One 512-token slice of the shared expert (bf16)."""
                st_eng = nc.sync if q == NTQ - 1 else nc.scalar
                last = q == NTQ - 1
                xq = xq_tiles[q]
                if q + 1 < NTQ and xq_tiles[q + 1] is None:
                    xq_tiles[q + 1] = xt_p.tile([128, NKD, TQ], bf16, tag="xt", name=f"xq{q+1}")
                    nc.sync.dma_start(xq_tiles[q + 1][:], xtq[q + 1])
                hsT = hst_p.tile([128, NSH, TQ], bf16, tag="hst")
                sgu = sg_slabs[0]
                for h in range(NSH):
                    pg = psA.tile([128, TQ], f32, tag="psA")
                    for k in range(NKD):
                        nc.tensor.matmul(pg[:], sgu[:, 0, h, k], xq[:, k],
                                         start=(k == 0), stop=(k == NKD - 1))
                    pu = psA.tile([128, TQ], f32, tag="psA")
                    for k in range(NKD):
                        nc.tensor.matmul(pu[:], sgu[:, 1, h, k], xq[:, k],
                                         start=(k == 0), stop=(k == NKD - 1))
                    tmp = tmp_p.tile([128, 512], bf16, tag="tmp")
                    nc.scalar.activation(tmp[:, :TQ], pg[:],
                                         mybir.ActivationFunctionType.Silu)
                    nc.vector.tensor_mul(hsT[:, h, :], tmp[:, :TQ], pu[:])
                for d in range(ND):
                    for ci in range(TQ // 128):
                        py = psB.tile([128, 512], f32, tag="psB")
                        for h in range(NSH):
                            nc.tensor.matmul(py[:], hsT[:, h, ci * 128:(ci + 1) * 128],
                                             sd_slabs[0][:, d, h],
                                             start=(h == 0), stop=(h == NSH - 1))
                        yst = y_p.tile([128, 512], bf16, tag="y")
                        nc.vector.tensor_copy(yst[:], py[:])
                        eng = (nc.sync if (d * 4 + ci) % 2 == 0 else nc.scalar) \
                            if last else st_eng
                        eng.dma_start(ys_out[q * 4 + ci, d], yst[:])

            # ---- routed experts, shared quarters interleaved as DMA slack ----
            def load_xg(j):
                chunks = []
                for g in range(NXG):
                    xc = xg_p.tile([128, KC, Cs[j]], fp8, tag="xg", name=f"xg{j}_{g}")
                    nc.sync.dma_start(xc[:], xgt_js[j][:, g * KC:(g + 1) * KC, :])
                    chunks.append(xc)
                return chunks

            def load_wgu_h0(j):
                gu = wgu_p.tile([128, 2, NKD, 128], fp8, tag="wgu", name=f"wgu{j}_h0")
                nc.sync.dma_start(gu[:, 0], wgu[j, 0, :, 0])
                nc.sync.dma_start(gu[:, 1], wgu[j, 0, :, 1])
                return gu

            def load_wgu_rest(j, gu0):
                slabs = [(gu0[:, 0], gu0[:, 1])]
                for h in range(1, NH):
                    gu = wgu_p.tile([128, 2, NKD, 128], fp8, tag="wgu")
                    nc.sync.dma_start(gu[:], wgu[j, h])
                    slabs.append((gu[:, 0], gu[:, 1]))
                return slabs

            xg_next = None
            slabs_next = None
            for j in range(E_LOC):
                C = Cs[j]
                if j == 0:
                    gu0 = load_wgu_h0(0)
                    xg_next = load_xg(0)
                    slabs_next = load_wgu_rest(0, gu0)
                w_slabs = slabs_next
                xg_chunks = xg_next
                hT = _expert_mlp(nc, (psA, tmp_p, ht_p),
                                 (xg_chunks, w_slabs), C, "ht", NH)
                if j == 0:
                    load_shared()
                # phase B': stationary = wd d-tiles (fp8 DoubleRow over h-pairs),
                # moving = hT tokens
                NCC = -(-C // 512)
                st_eng = nc.sync if j == E_LOC - 1 else nc.scalar
                for dh in range(2 * ND):
                    wd_s = wd_p.tile([128, NH, 2, 128], fp8, tag="wd")
                    nc.sync.dma_start(wd_s[:], wdd[j, dh])
                    for dt in range(2):
                        for cc in range(NCC):
                            w = min(512, C - cc * 512)
                            cs = slice(cc * 512, cc * 512 + w)
                            py = psB.tile([128, w], f32, tag="psB")
                            for hh in range(NHP):
                                nc.tensor.matmul(py[:],
                                                 wd_s[:, 2 * hh:2 * hh + 2, dt, :],
                                                 hT[:, 2 * hh:2 * hh + 2, cs],
                                                 start=(hh == 0), stop=False,
                                                 perf_mode=DR)
                            nc.tensor.matmul(py[:], wd_s[:, NH - 1, dt, :],
                                             hT[:, NH - 1, cs],
                                             start=False, stop=True)
                            yst = y_p.tile([128, 512], bf16, tag="y")
                            nc.vector.tensor_copy(yst[:, :w], py[:])
                            st_eng.dma_start(
                                y_outs[j][dh * 2 + dt, :, cs], yst[:, :w])
                if j + 1 < E_LOC:
                    xg_next = load_xg(j + 1)
                    gu0 = load_wgu_h0(j + 1)
                    slabs_next = load_wgu_rest(j + 1, gu0)
                shared_quarter(j)
    nc.compile()
    return nc


def kernel(hidden_states, gate_w, wg, wu, wd, swg, swu, swd):
    global LAST_RESULTS
    x = np.ascontiguousarray(np.asarray(hidden_states, np.float32).reshape(T, D))
    gate_w = np.asarray(gate_w, np.float32)
    wg = np.asarray(wg, np.float32)
    wu = np.asarray(wu, np.float32)
    wd = np.asarray(wd, np.float32)
    swg = np.asarray(swg, np.float32)
    swu = np.asarray(swu, np.float32)
    swd = np.asarray(swd, np.float32)

    # ---- host router ----
    topk_i, topk_w = _route(x, gate_w)
    idx = [np.where((topk_i == e).any(-1))[0] for e in range(E)]
    wts = [(topk_w * (topk_i == e))[idx[e]].sum(-1).astype(np.float32) / YDIV
           for e in range(E)]
    cnts = np.array([len(i) for i in idx])
    # bucket experts: slot j on every core serves similarly-loaded experts
    ranked = np.argsort(-cnts, kind="stable")            # expert ids, busiest first
    emap = ranked.reshape(E_LOC, N_CORES)                # emap[j, c] -> expert id
    Cs = [max(16, -(-int(cnts[emap[j]].max()) // 16) * 16) for j in range(E_LOC)]
    Cmax = max(Cs)

    nc = _build_bass(Cs)

    # ---- host shard + layout prep (all DMA sources partition-major) ----
    xT = np.ascontiguousarray(x.T)                      # [D, T] fp32
    xtq_np = np.ascontiguousarray(
        xT.reshape(NKD, 128, NTQ, TQ).transpose(2, 1, 0, 3).astype(BF16))

    in_maps = []
    for c in range(N_CORES):
        wgu_np = np.empty((E_LOC, NH, 128, 2, NKD, 128), F8)
        wdd_np = np.empty((E_LOC, 2 * ND, 128, NH, 2, 128), F8)
        xgt_nps = [np.zeros((128, NKD, Cs[j]), F8) for j in range(E_LOC)]
        for j in range(E_LOC):
            e = int(emap[j, c])
            wgu_np[j] = ((np.stack([wg[e], wu[e]]) * SWA)
                         .reshape(2, NKD, 128, NH, 128)
                         .transpose(3, 2, 0, 1, 4).astype(F8))
            wdd_np[j] = ((wd[e] * SWD).reshape(NH, 128, 2 * ND, 2, 128)
                         .transpose(2, 1, 0, 3, 4).astype(F8))
            cnt = int(cnts[e])
            xg = xT[:, idx[e]]                          # [D, cnt] fp32
            xgt_nps[j][:, :, :cnt] = (xg.reshape(NKD, 128, cnt)
                                      .transpose(1, 0, 2).astype(F8))
        sl = slice(c * HS_LOC, (c + 1) * HS_LOC)
        swg_c = np.zeros((D, HS_PAD), np.float32); swg_c[:, :HS_LOC] = swg[:, sl]
        swu_c = np.zeros((D, HS_PAD), np.float32); swu_c[:, :HS_LOC] = swu[:, sl]
        swd_c = np.zeros((HS_PAD, D), np.float32); swd_c[:HS_LOC] = swd[sl, :]
        swgu_np = (np.stack([swg_c, swu_c])
                   .reshape(2, NKD, 128, NSH, 128)
                   .transpose(2, 0, 3, 1, 4).astype(BF16))
        swdd_np = (swd_c.reshape(NSH, 128, ND, 512)
                   .transpose(1, 2, 0, 3).astype(BF16))
        im = {f"xgt{j}": xgt_nps[j] for j in range(E_LOC)}
        in_maps.append(im)
        im.update({
            "wgu": np.ascontiguousarray(wgu_np),
            "wdd": np.ascontiguousarray(wdd_np),
            "xtq": xtq_np,
            "swgu": np.ascontiguousarray(swgu_np),
            "swdd": np.ascontiguousarray(swdd_np),
        })

    res = run_bass_kernel_spmd(nc, in_maps, core_ids=list(range(N_CORES)))
    LAST_RESULTS = res

    # ---- host unshard: scatter-add routed outputs, sum shared partials ----
    out = np.zeros((T, D), np.float32)
    for c in range(N_CORES):
        ys = res.results[c]["ys_out"].astype(np.float32)  # [16, ND, 128, 512]
        out += ys.transpose(0, 2, 1, 3).reshape(T, D)
        for j in range(E_LOC):
            e = int(emap[j, c])
            cnt = int(cnts[e])
            y = res.results[c][f"y_out{j}"].reshape(D, Cs[j])[:, :cnt].astype(np.float32)
            out[idx[e]] += (y * wts[e][None, :]).T
    return out.reshape(B, S, D)
